# revision 1
# baseline (speedup 1.0000x reference)
"""Causal self-attention kernel for 8 Trainium2 NeuronCores.

Problem: B=4, T=2048, C=1024, NH=16, HD=64 (fp32).
Sharding: 8 cores = 4 batches x 2 head-groups (8 heads each).
Each core computes qkv projection + causal attention + its partial c_proj
for (batch b, heads hg*8..hg*8+7); host sums the two head-group partials.

On-device dataflow (per core, all matmuls float32r):
  x[b] --PE-transpose--> x^T --> q^T,k^T in [feat, T] layout (head-pair
  packed: 2 heads x 64 dims = 128 partitions) and v in [T, feat] layout
  augmented with a ones column per head (softmax denominator trick).
  S^T[k,q] = k^T.T @ q^T via two row-packed K=64 matmuls (tile_position);
  causal mask added with an identity matmul; exp on ScalarE over the
  [128,1024] two-head PSUM span; y~^T = v_aug.T @ P^T accumulated on PE
  (row 64 = denominator). Normalize with DVE reciprocal_approx_fast +
  GPSIMD partition_broadcast, then c_proj from y^T tiles.

Phase emission order P0 P1 A0 P2 C0 A1 P3 C1 A2 C2 A3 C3 keeps the
in-order PE queue from stalling on cross-phase dependency chains
(c_proj(c) needs the attention-epilogue normalize of chunk c; emitting
it two phases later hides that latency and keeps HAM warm).
"""

import math

import numpy as np

import concourse.bass as bass
import concourse.mybir as mybir
import concourse.tile as tile
from concourse import bacc
from concourse.bass_utils import run_bass_kernel_spmd

F32R = mybir.dt.float32r
F32 = mybir.dt.float32
EXP = mybir.ActivationFunctionType.Exp

B, T, C = 4, 2048, 1024
NH, HD = 16, 64
NHL = 8            # heads per core
PAIRS = 4          # head pairs per core
CH = 512           # q-chunk width
NCH = T // CH      # 4 q-chunks
KT = C // 128      # 8 contraction tiles over C
NTT = T // 128     # 16 T-tiles
SCALE = 1.0 / math.sqrt(HD)
NEG = -1.0e30


def build_nc():
    nc = bacc.Bacc("TRN2", target_bir_lowering=False)

    x_d = nc.dram_tensor("x_l", [T, C], F32R, kind="ExternalInput")
    wqk_d = nc.dram_tensor("w_qk", [1024, 1024], F32R, kind="ExternalInput")
    wv_d = nc.dram_tensor("w_v", [128, 4096], F32R, kind="ExternalInput")
    wp_d = nc.dram_tensor("w_p", [128, 4096], F32R, kind="ExternalInput")
    bqk_d = nc.dram_tensor("b_qk", [128, 8], F32, kind="ExternalInput")
    bv_d = nc.dram_tensor("b_v", [512], F32, kind="ExternalInput")
    bo_d = nc.dram_tensor("b_o", [C], F32, kind="ExternalInput")
    id_d = nc.dram_tensor("ident", [128, 128], F32R, kind="ExternalInput")
    mask_d = nc.dram_tensor("masks", [128, 1280], F32R, kind="ExternalInput")
    out_d = nc.dram_tensor("out_p", [T, C], F32, kind="ExternalOutput")

    with tile.TileContext(nc) as tc:
        with tc.tile_pool(name="cp", bufs=1) as cp, \
             tc.tile_pool(name="wk", bufs=1) as wk, \
             tc.tile_pool(name="ps", bufs=1, space="PSUM") as ps:
            # ---- constants (ident first: first transposes need only it) ----
            ident = cp.tile([128, 128], F32R, name="ident")
            nc.scalar.dma_start(ident, id_d.ap())
            # prefetch chunk-0 x tiles before the bulky constants
            xin0 = []
            for t4 in range(4):
                xi = wk.tile([128, C], F32R, tag="xin", bufs=4,
                             name=f"xin{t4}")
                nc.sync.dma_start(xi, x_d.ap()[t4 * 128:(t4 + 1) * 128, :])
                xin0.append(xi)
            bqk = cp.tile([128, 8], F32, name="bqk")
            nc.scalar.dma_start(bqk, bqk_d.ap())
            wv = cp.tile([128, 8, 512], F32R, name="wv")
            bv_row = cp.tile([1, 512], F32, name="bv_row")
            bv_rep = cp.tile([128, 512], F32, name="bv_rep")
            masks = cp.tile([128, 1280], F32R, name="masks")
            mask_off = {0: 0, 1: 128, 2: 384, 3: 768}
            bo_row = cp.tile([1, 1024], F32, name="bo_row")
            bo_rep = cp.tile([128, 1024], F32, name="bo_rep")
            wp = cp.tile([128, 4, 2, 512], F32R, name="wp")
            consts_loaded = set()

            def load_v_consts():
                if "v" in consts_loaded:
                    return
                consts_loaded.add("v")
                nc.scalar.dma_start(
                    wv, wv_d.ap().rearrange("p (a n) -> p a n", n=512))
                nc.scalar.dma_start(
                    bv_row, bv_d.ap().rearrange("(a n) -> a n", a=1))
                nc.gpsimd.partition_broadcast(bv_rep, bv_row)

            def load_a_consts():
                if "a" in consts_loaded:
                    return
                consts_loaded.add("a")
                nc.scalar.dma_start(masks, mask_d.ap())

            def load_c_consts():
                if "c" in consts_loaded:
                    return
                consts_loaded.add("c")
                nc.scalar.dma_start(
                    wp, wp_d.ap().rearrange("p (a b n) -> p a b n",
                                            a=4, b=2, n=512))
                nc.scalar.dma_start(
                    bo_row, bo_d.ap().rearrange("(a n) -> a n", a=1))
                nc.gpsimd.partition_broadcast(bo_rep, bo_row)

            # ---- persistent activations ----
            kT = [cp.tile([128, T], F32R, name=f"kT{p}") for p in range(PAIRS)]
            vt = cp.tile([128, NTT, 8 * 65], F32R, name="vt")

            qT = {}   # (pair, chunk) -> [128, 512] tile
            yT = {}   # (pair, chunk) -> [128, 512] tile
            XTS = {}  # chunk -> list of x^T tiles

            def proj_xt(c, xin_pre=None):
                ctx = nc.named_scope(f"xt{c}"); ctx.__enter__()
                xin = []
                for t4 in range(4):
                    tt = c * 4 + t4
                    if xin_pre is not None:
                        xi = xin_pre[t4]
                    else:
                        xi = wk.tile([128, C], F32R, tag="xin", bufs=4,
                                     name=f"xin{tt}")
                        nc.sync.dma_start(xi,
                                          x_d.ap()[tt * 128:(tt + 1) * 128, :])
                    xin.append(xi)
                xts = []
                for kc in range(KT):
                    xt_ps = ps.tile([128, 512], F32R, tag="pj", bufs=2,
                                    name=f"xtps{c}_{kc}")
                    for t4 in range(4):
                        nc.tensor.transpose(
                            xt_ps[:, t4 * 128:(t4 + 1) * 128],
                            xin[t4][:, kc * 128:(kc + 1) * 128], ident)
                    xt = wk.tile([128, 512], F32R, tag="xt", bufs=8,
                                 name=f"xt{c}_{kc}")
                    nc.vector.tensor_copy(xt, xt_ps)
                    xts.append(xt)
                XTS[c] = xts
                ctx.__exit__(None, None, None)

            def proj_qk(c, half):
                ctx = nc.named_scope(f"qk{c}_{half}"); ctx.__enter__()
                xts = XTS[c]
                for f in range(4 * half, 4 * half + 4):
                    wq = wk.tile([128, 8, 128], F32R, tag="wqk", bufs=2,
                                 name=f"wq{c}_{f}")
                    nc.scalar.dma_start(
                        wq, wqk_d.ap()[f * 128:(f + 1) * 128, :]
                        .rearrange("p (a j) -> p a j", j=128))
                    qk_ps = ps.tile([128, 512], F32, tag="pj", bufs=2,
                                    name=f"qkps{c}_{f}")
                    for kt in range(KT):
                        nc.tensor.matmul(qk_ps, wq[:, kt, :], xts[kt],
                                         start=(kt == 0), stop=(kt == KT - 1))
                    if f < 4:
                        qt = wk.tile([128, 512], F32R, tag="qT", bufs=7,
                                     name=f"qT{f}_{c}")
                        nc.vector.tensor_scalar_add(qt, qk_ps, bqk[:, f:f + 1])
                        qT[(f, c)] = qt
                    else:
                        nc.vector.tensor_scalar_add(
                            kT[f - 4][:, c * CH:(c + 1) * CH], qk_ps,
                            bqk[:, f:f + 1])
                ctx.__exit__(None, None, None)

            def proj_v(c):
                ctx = nc.named_scope(f"v{c}"); ctx.__enter__()
                load_v_consts()
                xts = XTS[c]
                for t4 in range(4):
                    tt = c * 4 + t4
                    v_ps = ps.tile([128, 512], F32, tag="pj", bufs=2,
                                   name=f"vps{tt}")
                    for kt in range(KT):
                        nc.tensor.matmul(v_ps, xts[kt][:, t4 * 128:(t4 + 1) * 128],
                                         wv[:, kt, :],
                                         start=(kt == 0), stop=(kt == KT - 1))
                    vslice = vt[:, tt, :].rearrange("p (h e) -> p h e", e=65)
                    nc.gpsimd.memset(
                        vt[:, tt, :].bitcast(F32)
                        .rearrange("p (h e) -> p h e", e=65)[:, :, 64:65], 1.0)
                    nc.vector.tensor_add(
                        vslice[:, :, 0:64],
                        v_ps.rearrange("p (h e) -> p h e", e=64),
                        bv_rep.rearrange("p (h e) -> p h e", e=64))
                del XTS[c]
                ctx.__exit__(None, None, None)

            def attn_pair(c, p):
                ctx = nc.named_scope(f"at{c}_{p}"); ctx.__enter__()
                load_a_consts()
                nkt = 4 * (c + 1)
                yA = ps.tile([65, 512], F32, tag="y", bufs=2,
                             name=f"yA{p}_{c}")
                yB = ps.tile([65, 512], F32, tag="y", bufs=2,
                             name=f"yB{p}_{c}")
                qtc = qT.pop((p, c))
                for kt in range(nkt):
                    s_ps = ps.tile([128, 1024], F32, tag="s", bufs=2,
                                   name=f"s{p}_{c}_{kt}")
                    d = kt * 128 - c * CH
                    partial = d >= 0
                    ksl = kT[p][:, kt * 128:(kt + 1) * 128]
                    nc.tensor.matmul(s_ps[:, 0:512], ksl[0:64, :],
                                     qtc[0:64, :], start=True,
                                     stop=not partial, tile_position=(0, 0))
                    nc.tensor.matmul(s_ps[:, 512:1024], ksl[64:128, :],
                                     qtc[64:128, :], start=True,
                                     stop=not partial,
                                     tile_position=(64, 0))
                    if partial:
                        n = d + 128
                        mo = mask_off[d // 128]
                        msl = masks[:, mo:mo + n]
                        nc.tensor.matmul(s_ps[:, 0:n], ident, msl,
                                         start=False, stop=True)
                        nc.tensor.matmul(s_ps[:, 512:512 + n], ident, msl,
                                         start=False, stop=True)
                    pt = wk.tile([128, 1024], F32R, tag="P", bufs=3,
                                 name=f"P{p}_{c}_{kt}")
                    nc.scalar.activation(pt, s_ps, EXP, scale=SCALE)
                    nc.tensor.matmul(
                        yA, vt[:, kt, (2 * p) * 65:(2 * p) * 65 + 65],
                        pt[:, 0:512],
                        start=(kt == 0), stop=(kt == nkt - 1))
                    nc.tensor.matmul(
                        yB, vt[:, kt, (2 * p + 1) * 65:(2 * p + 1) * 65 + 65],
                        pt[:, 512:1024],
                        start=(kt == 0), stop=(kt == nkt - 1))
                yt = wk.tile([128, 512], F32R, tag="yT", bufs=8,
                             name=f"yT{p}_{c}")
                for h, yps in ((0, yA), (1, yB)):
                    drow = wk.tile([1, 512], F32, tag="rc", bufs=2,
                                   name=f"dr{p}_{c}_{h}")
                    nc.vector.tensor_copy(drow, yps[64:65, :])
                    rc = wk.tile([1, 512], F32, tag="rc", bufs=2,
                                 name=f"rc{p}_{c}_{h}")
                    nc.vector.reciprocal_approx_fast(rc, drow)
                    rr = wk.tile([64, 512], F32, tag="rr", bufs=1,
                                 name=f"rr{p}_{c}_{h}")
                    nc.gpsimd.partition_broadcast(rr, rc)
                    nc.vector.tensor_mul(yt[h * 64:(h + 1) * 64, :],
                                         yps[0:64, :], rr)
                yT[(p, c)] = yt
                ctx.__exit__(None, None, None)

            def cproj_half(c, half):
                ctx = nc.named_scope(f"cp{c}_{half}"); ctx.__enter__()
                load_c_consts()
                for t4 in range(2 * half, 2 * half + 2):
                    tt = c * 4 + t4
                    for oc in range(2):
                        o_ps = ps.tile([128, 512], F32, tag="pj", bufs=2,
                                       name=f"ops{tt}_{oc}")
                        for p in range(PAIRS):
                            nc.tensor.matmul(
                                o_ps,
                                yT[(p, c)][:, t4 * 128:(t4 + 1) * 128],
                                wp[:, p, oc, :],
                                start=(p == 0), stop=(p == PAIRS - 1))
                        ot = wk.tile([128, 512], F32, tag="o", bufs=2,
                                     name=f"o{tt}_{oc}")
                        nc.vector.tensor_add(
                            ot, o_ps, bo_rep[:, oc * 512:(oc + 1) * 512])
                        nc.sync.dma_start(
                            out_d.ap()[tt * 128:(tt + 1) * 128,
                                       oc * 512:(oc + 1) * 512], ot)
                if half == 1:
                    for p in range(PAIRS):
                        yT.pop((p, c))
                ctx.__exit__(None, None, None)

            # fine-grained interleave: attention pairs alternate with
            # projection / c_proj slices so the in-order PE queue always has
            # exp-independent matmul work between ACT-dependent ones.
            proj_xt(0, xin_pre=xin0)
            proj_qk(0, 0)
            proj_qk(0, 1)
            proj_v(0)
            proj_xt(1)
            attn_pair(0, 0)
            proj_qk(1, 0)
            attn_pair(0, 1)
            proj_qk(1, 1)
            attn_pair(0, 2)
            proj_v(1)
            attn_pair(0, 3)
            proj_xt(2)
            attn_pair(1, 0)
            proj_qk(2, 0)
            attn_pair(1, 1)
            proj_qk(2, 1)
            attn_pair(1, 2)
            proj_v(2)
            attn_pair(1, 3)
            cproj_half(0, 0)
            attn_pair(2, 0)
            cproj_half(0, 1)
            attn_pair(2, 1)
            proj_xt(3)
            attn_pair(2, 2)
            proj_qk(3, 0)
            attn_pair(2, 3)
            proj_qk(3, 1)
            proj_v(3)
            cproj_half(1, 0)
            attn_pair(3, 0)
            cproj_half(1, 1)
            attn_pair(3, 1)
            cproj_half(2, 0)
            attn_pair(3, 2)
            cproj_half(2, 1)
            attn_pair(3, 3)
            cproj_half(3, 0)
            cproj_half(3, 1)

    nc.compile()
    return nc


_NC_CACHE = []


def _get_nc():
    if not _NC_CACHE:
        _NC_CACHE.append(build_nc())
    return _NC_CACHE[0]


def _host_consts():
    ident = np.eye(128, dtype=np.float32)
    kk = np.arange(128, dtype=np.int64)[:, None]
    masks = np.zeros((128, 1280), dtype=np.float32)
    off = 0
    for di in range(4):
        n = di * 128 + 128
        qq = np.arange(n, dtype=np.int64)[None, :]
        masks[:, off:off + n] = np.where(qq < kk + di * 128, NEG, 0.0)
        off += n
    return ident, masks


def _make_in_maps(x, W_attn, b_attn, W_proj, b_proj):
    ident, masks = _host_consts()
    in_maps = []
    for core in range(8):
        b, hg = core // 2, core % 2
        sl = slice(hg * 512, (hg + 1) * 512)
        w_q = W_attn[:, 0:1024][:, sl]
        w_k = W_attn[:, 1024:2048][:, sl]
        w_v = W_attn[:, 2048:3072][:, sl]
        in_maps.append({
            "x_l": np.ascontiguousarray(x[b]),
            "w_qk": np.ascontiguousarray(
                np.concatenate([w_q, w_k], axis=1).reshape(8, 128, 8, 128)
                .transpose(2, 1, 0, 3).reshape(1024, 1024)),
            "w_v": np.ascontiguousarray(
                w_v.reshape(8, 128, 512).transpose(1, 0, 2).reshape(128, 4096)),
            "w_p": np.ascontiguousarray(
                W_proj[sl, :].reshape(4, 128, 2, 512).transpose(1, 0, 2, 3)
                .reshape(128, 4096)),
            "b_qk": np.ascontiguousarray(
                np.concatenate([b_attn[0:1024][sl], b_attn[1024:2048][sl]])
                .reshape(8, 128).T),
            "b_v": np.ascontiguousarray(b_attn[2048:3072][sl]),
            "b_o": (b_proj if hg == 0
                    else np.zeros_like(b_proj)).astype(np.float32),
            "ident": ident,
            "masks": masks,
        })
    return in_maps


def _run(inputs, trace=False):
    x = np.asarray(inputs["x"], dtype=np.float32)
    W_attn = np.asarray(inputs["W_attn"], dtype=np.float32)
    b_attn = np.asarray(inputs["b_attn"], dtype=np.float32)
    W_proj = np.asarray(inputs["W_proj"], dtype=np.float32)
    b_proj = np.asarray(inputs["b_proj"], dtype=np.float32)

    nc = _get_nc()
    in_maps = _make_in_maps(x, W_attn, b_attn, W_proj, b_proj)
    res = run_bass_kernel_spmd(nc, in_maps, core_ids=list(range(8)),
                               trace=trace)
    out = np.empty((B, T, C), dtype=np.float32)
    for b in range(B):
        out[b] = res.results[2 * b]["out_p"] + res.results[2 * b + 1]["out_p"]
    return out, res


def kernel(**inputs) -> np.ndarray:
    out, _ = _run(inputs, trace=False)
    return out



# revision 3
# speedup vs baseline: 1.1799x; 1.1799x over previous
"""Causal self-attention kernel for 8 Trainium2 NeuronCores.

Problem: B=4, T=2048, C=1024, NH=16, HD=64 (fp32).
Sharding: 8 cores = 4 batches x 2 head-groups (8 heads each).
Each core computes qkv projection + causal attention + its partial c_proj
for (batch b, heads hg*8..hg*8+7); host sums the two head-group partials.

On-device dataflow (per core, all matmuls float32r):
  x[b] --PE-transpose--> x^T --> q^T,k^T in [feat, T] layout (head-pair
  packed: 2 heads x 64 dims = 128 partitions) and v in [T, feat] layout
  augmented with a ones column per head (softmax denominator trick).
  S^T[k,q] = k^T.T @ q^T via two row-packed K=64 matmuls (tile_position);
  causal mask added with an identity matmul; exp on ScalarE over the
  [128,1024] two-head PSUM span; y~^T = v_aug.T @ P^T accumulated on PE
  (row 64 = denominator). Normalize with DVE reciprocal_approx_fast +
  GPSIMD partition_broadcast, then c_proj from y^T tiles.

Phase emission order P0 P1 A0 P2 C0 A1 P3 C1 A2 C2 A3 C3 keeps the
in-order PE queue from stalling on cross-phase dependency chains
(c_proj(c) needs the attention-epilogue normalize of chunk c; emitting
it two phases later hides that latency and keeps HAM warm).
"""

import math

import numpy as np

import concourse.bass as bass
import concourse.mybir as mybir
import concourse.tile as tile
from concourse import bacc
from concourse.bass_utils import run_bass_kernel_spmd

F32R = mybir.dt.float32r
F32 = mybir.dt.float32
BF = mybir.dt.bfloat16
EXP = mybir.ActivationFunctionType.Exp

B, T, C = 4, 2048, 1024
NH, HD = 16, 64
NHL = 8            # heads per core
PAIRS = 4          # head pairs per core
CH = 512           # q-chunk width
NCH = T // CH      # 4 q-chunks
KT = C // 128      # 8 contraction tiles over C
NTT = T // 128     # 16 T-tiles
SCALE = 1.0 / math.sqrt(HD)
NEG = -1.0e30


def build_nc():
    nc = bacc.Bacc("TRN2", target_bir_lowering=False)

    x_d = nc.dram_tensor("x_l", [T, C], BF, kind="ExternalInput")
    wqk_d = nc.dram_tensor("w_qk", [1024, 1024], BF, kind="ExternalInput")
    wv_d = nc.dram_tensor("w_v", [128, 4096], BF, kind="ExternalInput")
    wp_d = nc.dram_tensor("w_p", [128, 4096], BF, kind="ExternalInput")
    bqk_d = nc.dram_tensor("b_qk", [128, 8], F32, kind="ExternalInput")
    bv_d = nc.dram_tensor("b_v", [512], F32, kind="ExternalInput")
    bo_d = nc.dram_tensor("b_o", [C], F32, kind="ExternalInput")
    id_d = nc.dram_tensor("ident", [128, 128], BF, kind="ExternalInput")
    mask_d = nc.dram_tensor("masks", [128, 1280], BF, kind="ExternalInput")
    out_d = nc.dram_tensor("out_p", [T, C], F32, kind="ExternalOutput")

    with tile.TileContext(nc) as tc:
        with tc.tile_pool(name="cp", bufs=1) as cp, \
             tc.tile_pool(name="wk", bufs=1) as wk, \
             tc.tile_pool(name="ps", bufs=1, space="PSUM") as ps:
            # ---- constants (ident first: first transposes need only it) ----
            ident = cp.tile([128, 128], BF, name="ident")
            nc.scalar.dma_start(ident, id_d.ap())
            # prefetch chunk-0 x tiles before the bulky constants
            xin0 = []
            for t4 in range(4):
                xi = wk.tile([128, C], BF, tag="xin", bufs=4,
                             name=f"xin{t4}")
                nc.sync.dma_start(xi, x_d.ap()[t4 * 128:(t4 + 1) * 128, :])
                xin0.append(xi)
            bqk = cp.tile([128, 8], F32, name="bqk")
            nc.scalar.dma_start(bqk, bqk_d.ap())
            wv = cp.tile([128, 8, 512], BF, name="wv")
            bv_row = cp.tile([1, 512], F32, name="bv_row")
            bv_rep = cp.tile([128, 512], F32, name="bv_rep")
            masks = cp.tile([128, 1280], BF, name="masks")
            mask_off = {0: 0, 1: 128, 2: 384, 3: 768}
            bo_row = cp.tile([1, 1024], F32, name="bo_row")
            bo_rep = cp.tile([128, 1024], F32, name="bo_rep")
            wp = cp.tile([128, 4, 2, 512], BF, name="wp")
            consts_loaded = set()

            def load_v_consts():
                if "v" in consts_loaded:
                    return
                consts_loaded.add("v")
                nc.scalar.dma_start(
                    wv, wv_d.ap().rearrange("p (a n) -> p a n", n=512))
                nc.scalar.dma_start(
                    bv_row, bv_d.ap().rearrange("(a n) -> a n", a=1))
                nc.gpsimd.partition_broadcast(bv_rep, bv_row)

            def load_a_consts():
                if "a" in consts_loaded:
                    return
                consts_loaded.add("a")
                nc.scalar.dma_start(masks, mask_d.ap())

            def load_c_consts():
                if "c" in consts_loaded:
                    return
                consts_loaded.add("c")
                nc.scalar.dma_start(
                    wp, wp_d.ap().rearrange("p (a b n) -> p a b n",
                                            a=4, b=2, n=512))
                nc.scalar.dma_start(
                    bo_row, bo_d.ap().rearrange("(a n) -> a n", a=1))
                nc.gpsimd.partition_broadcast(bo_rep, bo_row)

            # ---- persistent activations ----
            kT = [cp.tile([128, T], BF, name=f"kT{p}") for p in range(PAIRS)]
            vt = cp.tile([128, NTT, 8 * 65], BF, name="vt")

            qT = {}   # (pair, chunk) -> [128, 512] tile
            yT = {}   # (pair, chunk) -> [128, 512] tile
            XTS = {}  # chunk -> list of x^T tiles

            def proj_xt(c, xin_pre=None):
                ctx = nc.named_scope(f"xt{c}"); ctx.__enter__()
                xin = []
                for t4 in range(4):
                    tt = c * 4 + t4
                    if xin_pre is not None:
                        xi = xin_pre[t4]
                    else:
                        xi = wk.tile([128, C], BF, tag="xin", bufs=4,
                                     name=f"xin{tt}")
                        nc.sync.dma_start(xi,
                                          x_d.ap()[tt * 128:(tt + 1) * 128, :])
                    xin.append(xi)
                xts = []
                for kc in range(KT):
                    xt_ps = ps.tile([128, 512], BF, tag="pj", bufs=2,
                                    name=f"xtps{c}_{kc}")
                    for t4 in range(4):
                        nc.tensor.transpose(
                            xt_ps[:, t4 * 128:(t4 + 1) * 128],
                            xin[t4][:, kc * 128:(kc + 1) * 128], ident)
                    xt = wk.tile([128, 512], BF, tag="xt", bufs=8,
                                 name=f"xt{c}_{kc}")
                    nc.vector.tensor_copy(xt, xt_ps)
                    xts.append(xt)
                XTS[c] = xts
                ctx.__exit__(None, None, None)

            def proj_qk(c, half):
                ctx = nc.named_scope(f"qk{c}_{half}"); ctx.__enter__()
                xts = XTS[c]
                for f in range(4 * half, 4 * half + 4):
                    wq = wk.tile([128, 8, 128], BF, tag="wqk", bufs=2,
                                 name=f"wq{c}_{f}")
                    nc.scalar.dma_start(
                        wq, wqk_d.ap()[f * 128:(f + 1) * 128, :]
                        .rearrange("p (a j) -> p a j", j=128))
                    qk_ps = ps.tile([128, 512], F32, tag="pj", bufs=2,
                                    name=f"qkps{c}_{f}")
                    for kt in range(KT):
                        nc.tensor.matmul(qk_ps, wq[:, kt, :], xts[kt],
                                         start=(kt == 0), stop=(kt == KT - 1))
                    if f < 4:
                        qt = wk.tile([128, 512], BF, tag="qT", bufs=7,
                                     name=f"qT{f}_{c}")
                        nc.vector.tensor_scalar_add(qt, qk_ps, bqk[:, f:f + 1])
                        qT[(f, c)] = qt
                    else:
                        nc.vector.tensor_scalar_add(
                            kT[f - 4][:, c * CH:(c + 1) * CH], qk_ps,
                            bqk[:, f:f + 1])
                ctx.__exit__(None, None, None)

            def proj_v(c):
                ctx = nc.named_scope(f"v{c}"); ctx.__enter__()
                load_v_consts()
                xts = XTS[c]
                for t4 in range(4):
                    tt = c * 4 + t4
                    v_ps = ps.tile([128, 512], F32, tag="pj", bufs=2,
                                   name=f"vps{tt}")
                    for kt in range(KT):
                        nc.tensor.matmul(v_ps, xts[kt][:, t4 * 128:(t4 + 1) * 128],
                                         wv[:, kt, :],
                                         start=(kt == 0), stop=(kt == KT - 1))
                    vslice = vt[:, tt, :].rearrange("p (h e) -> p h e", e=65)
                    nc.gpsimd.memset(
                        vt[:, tt, :].rearrange("p (h e) -> p h e",
                                               e=65)[:, :, 64:65], 1.0)
                    nc.vector.tensor_add(
                        vslice[:, :, 0:64],
                        v_ps.rearrange("p (h e) -> p h e", e=64),
                        bv_rep.rearrange("p (h e) -> p h e", e=64))
                del XTS[c]
                ctx.__exit__(None, None, None)

            def attn_pair(c, p):
                ctx = nc.named_scope(f"at{c}_{p}"); ctx.__enter__()
                load_a_consts()
                nkt = 4 * (c + 1)
                yA = ps.tile([65, 512], F32, tag="y", bufs=2,
                             name=f"yA{p}_{c}")
                yB = ps.tile([65, 512], F32, tag="y", bufs=2,
                             name=f"yB{p}_{c}")
                qtc = qT.pop((p, c))
                for kt in range(nkt):
                    s_ps = ps.tile([128, 1024], F32, tag="s", bufs=2,
                                   name=f"s{p}_{c}_{kt}")
                    d = kt * 128 - c * CH
                    partial = d >= 0
                    ksl = kT[p][:, kt * 128:(kt + 1) * 128]
                    nc.tensor.matmul(s_ps[:, 0:512], ksl[0:64, :],
                                     qtc[0:64, :], start=True,
                                     stop=not partial, tile_position=(0, 0))
                    nc.tensor.matmul(s_ps[:, 512:1024], ksl[64:128, :],
                                     qtc[64:128, :], start=True,
                                     stop=not partial,
                                     tile_position=(64, 0))
                    if partial:
                        n = d + 128
                        mo = mask_off[d // 128]
                        msl = masks[:, mo:mo + n]
                        nc.tensor.matmul(s_ps[:, 0:n], ident, msl,
                                         start=False, stop=True)
                        nc.tensor.matmul(s_ps[:, 512:512 + n], ident, msl,
                                         start=False, stop=True)
                    pt = wk.tile([128, 1024], BF, tag="P", bufs=3,
                                 name=f"P{p}_{c}_{kt}")
                    nc.scalar.activation(pt, s_ps, EXP, scale=SCALE)
                    nc.tensor.matmul(
                        yA, vt[:, kt, (2 * p) * 65:(2 * p) * 65 + 65],
                        pt[:, 0:512],
                        start=(kt == 0), stop=(kt == nkt - 1))
                    nc.tensor.matmul(
                        yB, vt[:, kt, (2 * p + 1) * 65:(2 * p + 1) * 65 + 65],
                        pt[:, 512:1024],
                        start=(kt == 0), stop=(kt == nkt - 1))
                yt = wk.tile([128, 512], BF, tag="yT", bufs=8,
                             name=f"yT{p}_{c}")
                for h, yps in ((0, yA), (1, yB)):
                    drow = wk.tile([1, 512], F32, tag="rc", bufs=2,
                                   name=f"dr{p}_{c}_{h}")
                    nc.vector.tensor_copy(drow, yps[64:65, :])
                    rc = wk.tile([1, 512], F32, tag="rc", bufs=2,
                                 name=f"rc{p}_{c}_{h}")
                    nc.vector.reciprocal_approx_fast(rc, drow)
                    rr = wk.tile([64, 512], F32, tag="rr", bufs=1,
                                 name=f"rr{p}_{c}_{h}")
                    nc.gpsimd.partition_broadcast(rr, rc)
                    nc.vector.tensor_mul(yt[h * 64:(h + 1) * 64, :],
                                         yps[0:64, :], rr)
                yT[(p, c)] = yt
                ctx.__exit__(None, None, None)

            def cproj_half(c, half):
                ctx = nc.named_scope(f"cp{c}_{half}"); ctx.__enter__()
                load_c_consts()
                for t4 in range(2 * half, 2 * half + 2):
                    tt = c * 4 + t4
                    for oc in range(2):
                        o_ps = ps.tile([128, 512], F32, tag="pj", bufs=2,
                                       name=f"ops{tt}_{oc}")
                        for p in range(PAIRS):
                            nc.tensor.matmul(
                                o_ps,
                                yT[(p, c)][:, t4 * 128:(t4 + 1) * 128],
                                wp[:, p, oc, :],
                                start=(p == 0), stop=(p == PAIRS - 1))
                        ot = wk.tile([128, 512], F32, tag="o", bufs=2,
                                     name=f"o{tt}_{oc}")
                        nc.vector.tensor_add(
                            ot, o_ps, bo_rep[:, oc * 512:(oc + 1) * 512])
                        nc.sync.dma_start(
                            out_d.ap()[tt * 128:(tt + 1) * 128,
                                       oc * 512:(oc + 1) * 512], ot)
                if half == 1:
                    for p in range(PAIRS):
                        yT.pop((p, c))
                ctx.__exit__(None, None, None)

            # fine-grained interleave: attention pairs alternate with
            # projection / c_proj slices so the in-order PE queue always has
            # exp-independent matmul work between ACT-dependent ones.
            proj_xt(0, xin_pre=xin0)
            proj_qk(0, 0)
            proj_qk(0, 1)
            proj_v(0)
            proj_xt(1)
            attn_pair(0, 0)
            proj_qk(1, 0)
            attn_pair(0, 1)
            proj_qk(1, 1)
            attn_pair(0, 2)
            proj_v(1)
            attn_pair(0, 3)
            proj_xt(2)
            attn_pair(1, 0)
            proj_qk(2, 0)
            attn_pair(1, 1)
            proj_qk(2, 1)
            attn_pair(1, 2)
            proj_v(2)
            attn_pair(1, 3)
            cproj_half(0, 0)
            attn_pair(2, 0)
            cproj_half(0, 1)
            attn_pair(2, 1)
            proj_xt(3)
            attn_pair(2, 2)
            proj_qk(3, 0)
            attn_pair(2, 3)
            proj_qk(3, 1)
            proj_v(3)
            cproj_half(1, 0)
            attn_pair(3, 0)
            cproj_half(1, 1)
            attn_pair(3, 1)
            cproj_half(2, 0)
            attn_pair(3, 2)
            cproj_half(2, 1)
            attn_pair(3, 3)
            cproj_half(3, 0)
            cproj_half(3, 1)

    nc.compile()
    return nc


_NC_CACHE = []


def _get_nc():
    if not _NC_CACHE:
        _NC_CACHE.append(build_nc())
    return _NC_CACHE[0]


def _host_consts():
    ident = np.eye(128, dtype=np.float32)
    kk = np.arange(128, dtype=np.int64)[:, None]
    masks = np.zeros((128, 1280), dtype=np.float32)
    off = 0
    for di in range(4):
        n = di * 128 + 128
        qq = np.arange(n, dtype=np.int64)[None, :]
        masks[:, off:off + n] = np.where(qq < kk + di * 128, NEG, 0.0)
        off += n
    return ident, masks


def _make_in_maps(x, W_attn, b_attn, W_proj, b_proj):
    import ml_dtypes
    bf16 = ml_dtypes.bfloat16
    ident, masks = _host_consts()
    in_maps = []
    for core in range(8):
        b, hg = core // 2, core % 2
        sl = slice(hg * 512, (hg + 1) * 512)
        w_q = W_attn[:, 0:1024][:, sl]
        w_k = W_attn[:, 1024:2048][:, sl]
        w_v = W_attn[:, 2048:3072][:, sl]
        in_maps.append({
            "x_l": np.ascontiguousarray(x[b]).astype(bf16),
            "w_qk": np.ascontiguousarray(
                np.concatenate([w_q, w_k], axis=1).reshape(8, 128, 8, 128)
                .transpose(2, 1, 0, 3).reshape(1024, 1024)).astype(bf16),
            "w_v": np.ascontiguousarray(
                w_v.reshape(8, 128, 512).transpose(1, 0, 2)
                .reshape(128, 4096)).astype(bf16),
            "w_p": np.ascontiguousarray(
                W_proj[sl, :].reshape(4, 128, 2, 512).transpose(1, 0, 2, 3)
                .reshape(128, 4096)).astype(bf16),
            "b_qk": np.ascontiguousarray(
                np.concatenate([b_attn[0:1024][sl], b_attn[1024:2048][sl]])
                .reshape(8, 128).T),
            "b_v": np.ascontiguousarray(b_attn[2048:3072][sl]),
            "b_o": (b_proj if hg == 0
                    else np.zeros_like(b_proj)).astype(np.float32),
            "ident": ident.astype(bf16),
            "masks": masks.astype(bf16),
        })
    return in_maps


def _run(inputs, trace=False):
    x = np.asarray(inputs["x"], dtype=np.float32)
    W_attn = np.asarray(inputs["W_attn"], dtype=np.float32)
    b_attn = np.asarray(inputs["b_attn"], dtype=np.float32)
    W_proj = np.asarray(inputs["W_proj"], dtype=np.float32)
    b_proj = np.asarray(inputs["b_proj"], dtype=np.float32)

    nc = _get_nc()
    in_maps = _make_in_maps(x, W_attn, b_attn, W_proj, b_proj)
    res = run_bass_kernel_spmd(nc, in_maps, core_ids=list(range(8)),
                               trace=trace)
    out = np.empty((B, T, C), dtype=np.float32)
    for b in range(B):
        out[b] = res.results[2 * b]["out_p"] + res.results[2 * b + 1]["out_p"]
    return out, res


def kernel(**inputs) -> np.ndarray:
    out, _ = _run(inputs, trace=False)
    return out



# revision 5
# speedup vs baseline: 1.3256x; 1.1235x over previous
"""Causal self-attention kernel for 8 Trainium2 NeuronCores.

Problem: B=4, T=2048, C=1024, NH=16, HD=64 (fp32).
Sharding: 8 cores = 4 batches x 2 head-groups (8 heads each).
Each core computes qkv projection + causal attention + its partial c_proj
for (batch b, heads hg*8..hg*8+7); host sums the two head-group partials.

On-device dataflow (per core, all matmuls float32r):
  x[b] --PE-transpose--> x^T --> q^T,k^T in [feat, T] layout (head-pair
  packed: 2 heads x 64 dims = 128 partitions) and v in [T, feat] layout
  augmented with a ones column per head (softmax denominator trick).
  S^T[k,q] = k^T.T @ q^T via two row-packed K=64 matmuls (tile_position);
  causal mask added with an identity matmul; exp on ScalarE over the
  [128,1024] two-head PSUM span; y~^T = v_aug.T @ P^T accumulated on PE
  (row 64 = denominator). Normalize with DVE reciprocal_approx_fast +
  GPSIMD partition_broadcast, then c_proj from y^T tiles.

Phase emission order P0 P1 A0 P2 C0 A1 P3 C1 A2 C2 A3 C3 keeps the
in-order PE queue from stalling on cross-phase dependency chains
(c_proj(c) needs the attention-epilogue normalize of chunk c; emitting
it two phases later hides that latency and keeps HAM warm).
"""

import math

import numpy as np

import concourse.bass as bass
import concourse.mybir as mybir
import concourse.tile as tile
from concourse import bacc
from concourse.bass_utils import run_bass_kernel_spmd

F32R = mybir.dt.float32r
F32 = mybir.dt.float32
BF = mybir.dt.bfloat16
EXP = mybir.ActivationFunctionType.Exp

B, T, C = 4, 2048, 1024
NH, HD = 16, 64
NHL = 8            # heads per core
PAIRS = 4          # head pairs per core
CH = 512           # q-chunk width
NCH = T // CH      # 4 q-chunks
KT = C // 128      # 8 contraction tiles over C
NTT = T // 128     # 16 T-tiles
SCALE = 1.0 / math.sqrt(HD)
NEG = -1.0e30


def build_nc():
    nc = bacc.Bacc("TRN2", target_bir_lowering=False)

    x_d = nc.dram_tensor("x_l", [T, C], BF, kind="ExternalInput")
    wqk_d = nc.dram_tensor("w_qk", [1024, 1024], BF, kind="ExternalInput")
    wv_d = nc.dram_tensor("w_v", [128, 4096], BF, kind="ExternalInput")
    wp_d = nc.dram_tensor("w_p", [128, 4096], BF, kind="ExternalInput")
    bqk_d = nc.dram_tensor("b_qk", [128, 8], F32, kind="ExternalInput")
    bv_d = nc.dram_tensor("b_v", [512], F32, kind="ExternalInput")
    bo_d = nc.dram_tensor("b_o", [C], F32, kind="ExternalInput")
    id_d = nc.dram_tensor("ident", [128, 128], BF, kind="ExternalInput")
    mask_d = nc.dram_tensor("masks", [128, 128], BF, kind="ExternalInput")
    out_d = nc.dram_tensor("out_p", [T, C], F32, kind="ExternalOutput")

    with tile.TileContext(nc) as tc:
        with tc.tile_pool(name="cp", bufs=1) as cp, \
             tc.tile_pool(name="wk", bufs=1) as wk, \
             tc.tile_pool(name="ps", bufs=1, space="PSUM") as ps:
            # ---- constants (ident first: first transposes need only it) ----
            ident = cp.tile([128, 128], BF, name="ident")
            nc.scalar.dma_start(ident, id_d.ap())
            # prefetch chunk-0 x tiles before the bulky constants
            xin0 = []
            for t4 in range(4):
                xi = wk.tile([128, C], BF, tag="xin", bufs=4,
                             name=f"xin{t4}")
                nc.sync.dma_start(xi, x_d.ap()[t4 * 128:(t4 + 1) * 128, :])
                xin0.append(xi)
            bqk = cp.tile([128, 8], F32, name="bqk")
            nc.scalar.dma_start(bqk, bqk_d.ap())
            wv = cp.tile([128, 8, 512], BF, name="wv")
            bv_row = cp.tile([1, 512], F32, name="bv_row")
            bv_rep = cp.tile([128, 512], F32, name="bv_rep")
            masks = cp.tile([128, 128], BF, name="masks")
            bo_row = cp.tile([1, 1024], F32, name="bo_row")
            bo_rep = cp.tile([128, 1024], F32, name="bo_rep")
            wp = cp.tile([128, 4, 2, 512], BF, name="wp")
            consts_loaded = set()

            def load_v_consts():
                if "v" in consts_loaded:
                    return
                consts_loaded.add("v")
                nc.scalar.dma_start(
                    wv, wv_d.ap().rearrange("p (a n) -> p a n", n=512))
                nc.scalar.dma_start(
                    bv_row, bv_d.ap().rearrange("(a n) -> a n", a=1))
                nc.gpsimd.partition_broadcast(bv_rep, bv_row)

            def load_a_consts():
                if "a" in consts_loaded:
                    return
                consts_loaded.add("a")
                nc.scalar.dma_start(masks, mask_d.ap())

            def load_c_consts():
                if "c" in consts_loaded:
                    return
                consts_loaded.add("c")
                nc.scalar.dma_start(
                    wp, wp_d.ap().rearrange("p (a b n) -> p a b n",
                                            a=4, b=2, n=512))
                nc.scalar.dma_start(
                    bo_row, bo_d.ap().rearrange("(a n) -> a n", a=1))
                nc.gpsimd.partition_broadcast(bo_rep, bo_row)

            # ---- persistent activations ----
            kT = [cp.tile([128, T], BF, name=f"kT{p}") for p in range(PAIRS)]
            vt = cp.tile([128, NTT, 8 * 65], BF, name="vt")

            qT = {}   # (pair, chunk) -> [128, 512] tile
            yT = {}   # (pair, chunk) -> [128, 512] tile
            XTS = {}  # chunk -> list of x^T tiles

            def proj_xt(c, xin_pre=None):
                ctx = nc.named_scope(f"xt{c}"); ctx.__enter__()
                xin = []
                for t4 in range(4):
                    tt = c * 4 + t4
                    if xin_pre is not None:
                        xi = xin_pre[t4]
                    else:
                        xi = wk.tile([128, C], BF, tag="xin", bufs=4,
                                     name=f"xin{tt}")
                        nc.sync.dma_start(xi,
                                          x_d.ap()[tt * 128:(tt + 1) * 128, :])
                    xin.append(xi)
                xts = []
                for kc in range(KT):
                    xt_ps = ps.tile([128, 512], BF, tag="pj", bufs=2,
                                    name=f"xtps{c}_{kc}")
                    for t4 in range(4):
                        nc.tensor.transpose(
                            xt_ps[:, t4 * 128:(t4 + 1) * 128],
                            xin[t4][:, kc * 128:(kc + 1) * 128], ident)
                    xt = wk.tile([128, 512], BF, tag="xt", bufs=8,
                                 name=f"xt{c}_{kc}")
                    nc.vector.tensor_copy(xt, xt_ps)
                    xts.append(xt)
                XTS[c] = xts
                ctx.__exit__(None, None, None)

            def proj_qk(c, half):
                ctx = nc.named_scope(f"qk{c}_{half}"); ctx.__enter__()
                xts = XTS[c]
                for f in range(4 * half, 4 * half + 4):
                    wq = wk.tile([128, 8, 128], BF, tag="wqk", bufs=2,
                                 name=f"wq{c}_{f}")
                    nc.scalar.dma_start(
                        wq, wqk_d.ap()[f * 128:(f + 1) * 128, :]
                        .rearrange("p (a j) -> p a j", j=128))
                    qk_ps = ps.tile([128, 512], F32, tag="pj", bufs=2,
                                    name=f"qkps{c}_{f}")
                    for kt in range(KT):
                        nc.tensor.matmul(qk_ps, wq[:, kt, :], xts[kt],
                                         start=(kt == 0), stop=(kt == KT - 1))
                    if f < 4:
                        qt = wk.tile([128, 512], BF, tag="qT", bufs=7,
                                     name=f"qT{f}_{c}")
                        nc.vector.tensor_scalar_add(qt, qk_ps, bqk[:, f:f + 1])
                        qT[(f, c)] = qt
                    else:
                        nc.vector.tensor_scalar_add(
                            kT[f - 4][:, c * CH:(c + 1) * CH], qk_ps,
                            bqk[:, f:f + 1])
                ctx.__exit__(None, None, None)

            def proj_v(c):
                ctx = nc.named_scope(f"v{c}"); ctx.__enter__()
                load_v_consts()
                xts = XTS[c]
                for t4 in range(4):
                    tt = c * 4 + t4
                    v_ps = ps.tile([128, 512], F32, tag="pj", bufs=2,
                                   name=f"vps{tt}")
                    for kt in range(KT):
                        nc.tensor.matmul(v_ps, xts[kt][:, t4 * 128:(t4 + 1) * 128],
                                         wv[:, kt, :],
                                         start=(kt == 0), stop=(kt == KT - 1))
                    vslice = vt[:, tt, :].rearrange("p (h e) -> p h e", e=65)
                    nc.gpsimd.memset(
                        vt[:, tt, :].rearrange("p (h e) -> p h e",
                                               e=65)[:, :, 64:65], 1.0)
                    nc.vector.tensor_add(
                        vslice[:, :, 0:64],
                        v_ps.rearrange("p (h e) -> p h e", e=64),
                        bv_rep.rearrange("p (h e) -> p h e", e=64))
                del XTS[c]
                ctx.__exit__(None, None, None)

            def attn_pair(c, p):
                ctx = nc.named_scope(f"at{c}_{p}"); ctx.__enter__()
                load_a_consts()
                nkt = 4 * (c + 1)
                yA = ps.tile([65, 512], F32, tag="y", bufs=2,
                             name=f"yA{p}_{c}")
                yB = ps.tile([65, 512], F32, tag="y", bufs=2,
                             name=f"yB{p}_{c}")
                qtc = qT.pop((p, c))
                for kt in range(nkt):
                    s_ps = ps.tile([128, 1024], F32, tag="s", bufs=2,
                                   name=f"s{p}_{c}_{kt}")
                    partial = kt * 128 - c * CH >= 0
                    d = max(kt * 128 - c * CH, 0)
                    ksl = kT[p][:, kt * 128:(kt + 1) * 128]
                    nc.tensor.matmul(s_ps[:, d:512], ksl[0:64, :],
                                     qtc[0:64, d:512], start=True,
                                     stop=not partial, tile_position=(0, 0))
                    nc.tensor.matmul(s_ps[:, 512 + d:1024], ksl[64:128, :],
                                     qtc[64:128, d:512], start=True,
                                     stop=not partial,
                                     tile_position=(64, 0))
                    if partial:
                        msl = masks[:, 0:128]
                        nc.tensor.matmul(s_ps[:, d:d + 128], ident, msl,
                                         start=False, stop=True)
                        nc.tensor.matmul(s_ps[:, 512 + d:512 + d + 128],
                                         ident, msl,
                                         start=False, stop=True)
                    pt = wk.tile([128, 1024], BF, tag="P", bufs=3,
                                 name=f"P{p}_{c}_{kt}")
                    if d > 0:
                        s_v = s_ps.rearrange("p (h q) -> p h q",
                                             q=512)[:, :, d:512]
                        p_v = pt.rearrange("p (h q) -> p h q",
                                           q=512)[:, :, d:512]
                        nc.scalar.activation(p_v, s_v, EXP, scale=SCALE)
                    else:
                        nc.scalar.activation(pt, s_ps, EXP, scale=SCALE)
                    nc.tensor.matmul(
                        yA[:, d:512],
                        vt[:, kt, (2 * p) * 65:(2 * p) * 65 + 65],
                        pt[:, d:512],
                        start=(kt == 0), stop=(kt == nkt - 1))
                    nc.tensor.matmul(
                        yB[:, d:512],
                        vt[:, kt, (2 * p + 1) * 65:(2 * p + 1) * 65 + 65],
                        pt[:, 512 + d:1024],
                        start=(kt == 0), stop=(kt == nkt - 1))
                yt = wk.tile([128, 512], BF, tag="yT", bufs=8,
                             name=f"yT{p}_{c}")
                for h, yps in ((0, yA), (1, yB)):
                    drow = wk.tile([1, 512], F32, tag="rc", bufs=2,
                                   name=f"dr{p}_{c}_{h}")
                    nc.vector.tensor_copy(drow, yps[64:65, :])
                    rc = wk.tile([1, 512], F32, tag="rc", bufs=2,
                                 name=f"rc{p}_{c}_{h}")
                    nc.vector.reciprocal_approx_fast(rc, drow)
                    rr = wk.tile([64, 512], F32, tag="rr", bufs=1,
                                 name=f"rr{p}_{c}_{h}")
                    nc.gpsimd.partition_broadcast(rr, rc)
                    nc.vector.tensor_mul(yt[h * 64:(h + 1) * 64, :],
                                         yps[0:64, :], rr)
                yT[(p, c)] = yt
                ctx.__exit__(None, None, None)

            def cproj_half(c, half):
                ctx = nc.named_scope(f"cp{c}_{half}"); ctx.__enter__()
                load_c_consts()
                for t4 in range(2 * half, 2 * half + 2):
                    tt = c * 4 + t4
                    for oc in range(2):
                        o_ps = ps.tile([128, 512], F32, tag="pj", bufs=2,
                                       name=f"ops{tt}_{oc}")
                        for p in range(PAIRS):
                            nc.tensor.matmul(
                                o_ps,
                                yT[(p, c)][:, t4 * 128:(t4 + 1) * 128],
                                wp[:, p, oc, :],
                                start=(p == 0), stop=(p == PAIRS - 1))
                        ot = wk.tile([128, 512], F32, tag="o", bufs=2,
                                     name=f"o{tt}_{oc}")
                        nc.vector.tensor_add(
                            ot, o_ps, bo_rep[:, oc * 512:(oc + 1) * 512])
                        nc.sync.dma_start(
                            out_d.ap()[tt * 128:(tt + 1) * 128,
                                       oc * 512:(oc + 1) * 512], ot)
                if half == 1:
                    for p in range(PAIRS):
                        yT.pop((p, c))
                ctx.__exit__(None, None, None)

            # fine-grained interleave: attention pairs alternate with
            # projection / c_proj slices so the in-order PE queue always has
            # exp-independent matmul work between ACT-dependent ones.
            proj_xt(0, xin_pre=xin0)
            proj_qk(0, 0)
            proj_qk(0, 1)
            proj_v(0)
            proj_xt(1)
            attn_pair(0, 0)
            proj_qk(1, 0)
            attn_pair(0, 1)
            proj_qk(1, 1)
            attn_pair(0, 2)
            proj_v(1)
            attn_pair(0, 3)
            proj_xt(2)
            attn_pair(1, 0)
            proj_qk(2, 0)
            attn_pair(1, 1)
            proj_qk(2, 1)
            attn_pair(1, 2)
            proj_v(2)
            attn_pair(1, 3)
            cproj_half(0, 0)
            attn_pair(2, 0)
            cproj_half(0, 1)
            attn_pair(2, 1)
            proj_xt(3)
            attn_pair(2, 2)
            proj_qk(3, 0)
            attn_pair(2, 3)
            proj_qk(3, 1)
            proj_v(3)
            cproj_half(1, 0)
            attn_pair(3, 0)
            cproj_half(1, 1)
            attn_pair(3, 1)
            cproj_half(2, 0)
            attn_pair(3, 2)
            cproj_half(2, 1)
            attn_pair(3, 3)
            cproj_half(3, 0)
            cproj_half(3, 1)

    nc.compile()
    return nc


_NC_CACHE = []


def _get_nc():
    if not _NC_CACHE:
        _NC_CACHE.append(build_nc())
    return _NC_CACHE[0]


def _host_consts():
    ident = np.eye(128, dtype=np.float32)
    kk = np.arange(128, dtype=np.int64)[:, None]
    qq = np.arange(128, dtype=np.int64)[None, :]
    masks = np.where(qq < kk, NEG, 0.0).astype(np.float32)
    return ident, masks


def _make_in_maps(x, W_attn, b_attn, W_proj, b_proj):
    import ml_dtypes
    bf16 = ml_dtypes.bfloat16
    ident, masks = _host_consts()
    in_maps = []
    for core in range(8):
        b, hg = core // 2, core % 2
        sl = slice(hg * 512, (hg + 1) * 512)
        w_q = W_attn[:, 0:1024][:, sl]
        w_k = W_attn[:, 1024:2048][:, sl]
        w_v = W_attn[:, 2048:3072][:, sl]
        in_maps.append({
            "x_l": np.ascontiguousarray(x[b]).astype(bf16),
            "w_qk": np.ascontiguousarray(
                np.concatenate([w_q, w_k], axis=1).reshape(8, 128, 8, 128)
                .transpose(2, 1, 0, 3).reshape(1024, 1024)).astype(bf16),
            "w_v": np.ascontiguousarray(
                w_v.reshape(8, 128, 512).transpose(1, 0, 2)
                .reshape(128, 4096)).astype(bf16),
            "w_p": np.ascontiguousarray(
                W_proj[sl, :].reshape(4, 128, 2, 512).transpose(1, 0, 2, 3)
                .reshape(128, 4096)).astype(bf16),
            "b_qk": np.ascontiguousarray(
                np.concatenate([b_attn[0:1024][sl], b_attn[1024:2048][sl]])
                .reshape(8, 128).T),
            "b_v": np.ascontiguousarray(b_attn[2048:3072][sl]),
            "b_o": (b_proj if hg == 0
                    else np.zeros_like(b_proj)).astype(np.float32),
            "ident": ident.astype(bf16),
            "masks": masks.astype(bf16),
        })
    return in_maps


def _run(inputs, trace=False):
    x = np.asarray(inputs["x"], dtype=np.float32)
    W_attn = np.asarray(inputs["W_attn"], dtype=np.float32)
    b_attn = np.asarray(inputs["b_attn"], dtype=np.float32)
    W_proj = np.asarray(inputs["W_proj"], dtype=np.float32)
    b_proj = np.asarray(inputs["b_proj"], dtype=np.float32)

    nc = _get_nc()
    in_maps = _make_in_maps(x, W_attn, b_attn, W_proj, b_proj)
    res = run_bass_kernel_spmd(nc, in_maps, core_ids=list(range(8)),
                               trace=trace)
    out = np.empty((B, T, C), dtype=np.float32)
    for b in range(B):
        out[b] = res.results[2 * b]["out_p"] + res.results[2 * b + 1]["out_p"]
    return out, res


def kernel(**inputs) -> np.ndarray:
    out, _ = _run(inputs, trace=False)
    return out

